# revision 28
# baseline (speedup 1.0000x reference)
"""AWQ 4-bit quantized linear layer on 8 Trainium2 NeuronCores.

Problem: out = x @ dequant(qweight, scales, qzeros) + bias
  x [8192,4096] f16, qweight [4096,1536] i32 (int4 nibbles), scales [32,12288]
  f16 (group 128), qzeros [32,1536] i32, bias [12288] f16 -> out [8192,12288].

Sharding: tensor-parallel colwise, 8 shards of 1536 out-features, x
replicated (host-transposed); host concatenates; no collectives.

The kernel is PE-bound: 6144 bf16 matmuls (K=128, N=512) at the 216 ns
issue floor = 1.327 ms. The structure exists to keep the PE fed from the
first microsecond despite the dequant startup being DMA-bound (aggregate
HBM-DMA cap ~343 GB/s/core; the per-k-tile [scale|zero] row broadcasts are
128x-fanout traffic):
  1. Host layout prep: qweight nibbles widened to uint8 (no arithmetic),
     x pre-transposed, metadata rows [s | -z*s] precomputed (0.4% of
     bytes), plus xg0 = per-group column sums of the first 512 x rows.
  2. Wave 1 dequant produces w' = wq*s ONLY (zero-point deferred), halving
     broadcast traffic in the critical window: per k-tile pair one qw DMA,
     one s-row broadcast, uint8->f16 cast on ACT, one DVE mult. W resident.
  3. Superchunk 0 runs k-PHASED against the dequant frontier (phase A: 8
     psum accumulators over o=0,1; phase B: o=2, t-outer). Each
     accumulator gets one extra K=32 matmul x_groupsums @ (-z*s) that
     exactly cancels the deferred zero-point term (out = x@w' - xg@zs).
  4. Wave 2 (during phase B, on the now-idle rings) broadcasts -z*s and
     adds it into W in place; phase B is t-outer so each w-tile's last
     un-subtracted read completes early. Superchunk 1 is also k-phased so
     it consumes w-tiles no faster than the subs complete. Superchunks
     2..15 then run at the full 216 ns/MM rate with fully-dequanted W.
"""

import sys

for p in ("/opt/trn_rl_repo", "/opt/pypackages"):
    if p not in sys.path:
        sys.path.insert(0, p)

import numpy as np

import concourse.bacc as bacc
import concourse.bass as bass
import concourse.mybir as mybir
from concourse.tile import TileContext

f16 = mybir.dt.float16
f32 = mybir.dt.float32
i16 = mybir.dt.int16
Alu = mybir.AluOpType

N_CORES = 8
M_FULL, K_FULL, O_FULL = 8192, 4096, 12288
GROUP_SIZE = 128
PACK = 8

O_SHARD = O_FULL // N_CORES        # 1536
C_SHARD = O_SHARD // PACK          # 192


def _perm(C):
    """Per-core column permutation for the int16 nibble unpack: permuted
    position jj*(2C) + 2c + h holds the natural out-feature 8c + 4h + jj.
    Each of the 4 device unpack ops writes one contiguous 2C-wide block."""
    p2f = np.empty(PACK * C, dtype=np.int64)
    for jj in range(4):
        for c in range(C):
            for h in range(2):
                p2f[jj * 2 * C + 2 * c + h] = 8 * c + 4 * h + jj
    return p2f


def build_nc(M=M_FULL, K=K_FULL, O=O_SHARD, MS=512, xt_bufs=20):
    KT = K // 128                  # k-tiles == quant groups (32)
    G = K // GROUP_SIZE
    assert KT == G
    OT = O // 512                  # 3
    NMS = M // MS                  # 16
    MT = MS // 128                 # 4
    NP = KT // 2                   # 16 k-tile pairs

    C2 = O // 4                    # int16 columns per shard (384)
    nc = bacc.Bacc("TRN2")
    xt_in = nc.dram_tensor("xt", [K, M], f16, kind="ExternalInput")
    qw16 = nc.dram_tensor("qw16", [K, C2], i16, kind="ExternalInput")
    # [:, :O] = s, [:, O:] = -z*s (negated so wave-2 and the correction
    # matmul both ADD)
    ssz = nc.dram_tensor("ssz", [G, 2 * O], f16, kind="ExternalInput")
    xg0_in = nc.dram_tensor("xg0", [G, MS], f16, kind="ExternalInput")
    bias = nc.dram_tensor("bias", [1, O], f16, kind="ExternalInput")
    out = nc.dram_tensor("out", [M, O], f16, kind="ExternalOutput")

    with TileContext(nc) as tc:
        with (
            tc.tile_pool(name="wres", bufs=NP) as w_pool,
            tc.tile_pool(name="xt", bufs=xt_bufs) as xt_pool,
            tc.tile_pool(name="qwc", bufs=2) as qwc_pool,
            tc.tile_pool(name="bc", bufs=2) as bc_pool,
            tc.tile_pool(name="meta", bufs=1) as meta_pool,
            tc.tile_pool(name="obuf", bufs=4) as o_pool,
            tc.tile_pool(name="psum", bufs=8, space="PSUM") as psum_pool,
        ):
            qw_r = qw16.rearrange("(t p) c -> p t c", p=128)
            xt_r = xt_in.rearrange("(t p) m -> p t m", p=128)

            # ---- wave 1: w' = wq * s, resident; zero-point deferred ----
            w_tiles = []
            w_pairs = []
            xts0 = []
            for u in range(NP):
                qw_c = qwc_pool.tile([128, 2, C2], i16, tag="qwc", name="qwc")
                nc.sync.dma_start(qw_c[:], qw_r[:, 2 * u:2 * u + 2, :])
                xtp = xt_pool.tile([128, 2, MS], f16, tag="xt", name="xt")
                nc.sync.dma_start(xtp[:], xt_r[:, 2 * u:2 * u + 2, 0:MS])
                xts0.append(xtp[:, 0, :])
                xts0.append(xtp[:, 1, :])
                s_b = bc_pool.tile([128, 2, O], f16, tag="sb", name="s_b")
                nc.scalar.dma_start(
                    s_b[:], ssz[2 * u:2 * u + 2, :O].partition_broadcast(128))
                # int16 nibble unpack on DVE (it has slack in wave 1)
                wq_i = qwc_pool.tile([128, 2, O], i16, tag="wqi", name="wqi")
                for jj in range(4):
                    nc.vector.tensor_scalar(
                        wq_i[:, :, jj * C2:(jj + 1) * C2],
                        qw_c[:], 4 * jj, 0xF,
                        Alu.logical_shift_right, Alu.bitwise_and,
                    )
                wq_f = bc_pool.tile([128, 2, O], f16, tag="wqf", name="wqf")
                nc.scalar.copy(wq_f[:], wq_i[:])
                w2 = w_pool.tile([128, 2, O], f16, tag="w", name="w")
                nc.vector.tensor_tensor(w2[:], wq_f[:], s_b[:], Alu.mult)
                w_pairs.append(w2)
                for v in range(2):
                    w_tiles.append(w2[:, v, :])
                if u == 2:
                    # off the startup critical path, before first evict/corr
                    bias_b = meta_pool.tile([128, O], f16, tag="biasb")
                    nc.scalar.dma_start(
                        bias_b[:], bias[0, :].partition_broadcast(128))
                    xg0_sb = meta_pool.tile([G, MS], f16, tag="xg0")
                    nc.scalar.dma_start(xg0_sb[:], xg0_in[:, :])
                    zsn_sb = meta_pool.tile([G, O], f16, tag="zsn")
                    nc.scalar.dma_start(zsn_sb[:], ssz[:, O:])

            def corr_mm(ps, mi, o):
                """K=32 matmul adding -(xg0 @ z*s): cancels the deferred
                zero-point for superchunk 0. Ends the accumulation group."""
                nc.tensor.matmul(
                    ps[:],
                    xg0_sb[:, mi * 128:(mi + 1) * 128],
                    zsn_sb[:, o * 512:(o + 1) * 512],
                    start=False, stop=True, skip_group_check=True,
                )

            def evict(ps, mi, o, ms_base, ring):
                ob = o_pool.tile([128, 512], f16, tag="ob", name="ob")
                nc.scalar.copy(ob[:], ps[:])
                nc.vector.tensor_tensor(
                    ob[:], ob[:], bias_b[:, o * 512:(o + 1) * 512], Alu.add)
                m0 = ms_base + mi * 128
                ring.dma_start(
                    out[m0:m0 + 128, o * 512:(o + 1) * 512], ob[:])

            # ---- superchunk 0, phase A: o=0,1 k-phased on 8 banks ----
            psA = []
            for mi in range(MT):
                for o in range(2):
                    ps = psum_pool.tile([128, 512], f32, tag="ps", name="ps")
                    psA.append((mi, o, ps))
            for t in range(KT):
                for mi, o, ps in psA:
                    nc.tensor.matmul(
                        ps[:],
                        xts0[t][:, mi * 128:(mi + 1) * 128],
                        w_tiles[t][:, o * 512:(o + 1) * 512],
                        start=(t == 0), stop=False,
                        skip_group_check=True,
                    )
            for mi, o, ps in psA:
                corr_mm(ps, mi, o)
                evict(ps, mi, o, 0, nc.sync)

            # wave-2 zs broadcasts prefire here so they stream on the ACT
            # ring DURING phase B instead of queueing behind its evict
            # copies (engine queues are strict FIFO); buffer-gated to 3
            zs_bufs = []
            for u in range(NP):
                zs_b = bc_pool.tile([128, 2, O], f16, tag="zsb",
                                    name="zs_b", bufs=3)
                nc.scalar.dma_start(
                    zs_b[:], ssz[2 * u:2 * u + 2, O:].partition_broadcast(128))
                zs_bufs.append(zs_b)

            # ---- superchunk 0, phase B: o=2, t-outer so each w-tile's
            # last pre-subtract read completes early ----
            psB = []
            for mi in range(MT):
                ps = psum_pool.tile([128, 512], f32, tag="ps", name="ps")
                psB.append((mi, ps))
            for t in range(KT):
                for mi, ps in psB:
                    nc.tensor.matmul(
                        ps[:],
                        xts0[t][:, mi * 128:(mi + 1) * 128],
                        w_tiles[t][:, 2 * 512:3 * 512],
                        start=(t == 0), stop=False,
                        skip_group_check=True,
                    )
            for mi, ps in psB:
                corr_mm(ps, mi, 2)
                evict(ps, mi, 2, 0, nc.sync)

            # ---- wave 2: w += (-z*s); superchunk-1 x interleaves on SP ----
            xts1 = []
            for u in range(NP):
                xtp = xt_pool.tile([128, 2, MS], f16, tag="xt", name="xt")
                nc.sync.dma_start(xtp[:], xt_r[:, 2 * u:2 * u + 2, MS:2 * MS])
                xts1.append(xtp[:, 0, :])
                xts1.append(xtp[:, 1, :])
                nc.vector.tensor_tensor(
                    w_pairs[u][:], w_pairs[u][:], zs_bufs[u][:], Alu.add)

            # ---- superchunk 1: k-phased (consumes w no faster than the
            # wave-2 subs complete), then o=2 normally ----
            psC = []
            for mi in range(MT):
                for o in range(2):
                    ps = psum_pool.tile([128, 512], f32, tag="ps", name="ps")
                    psC.append((mi, o, ps))
            for t in range(KT):
                for mi, o, ps in psC:
                    nc.tensor.matmul(
                        ps[:],
                        xts1[t][:, mi * 128:(mi + 1) * 128],
                        w_tiles[t][:, o * 512:(o + 1) * 512],
                        start=(t == 0), stop=(t == KT - 1),
                        skip_group_check=True,
                    )
            for mi, o, ps in psC:
                evict(ps, mi, o, MS, nc.sync)
            for mi in range(MT):
                ps = psum_pool.tile([128, 512], f32, tag="ps", name="ps")
                for t in range(KT):
                    nc.tensor.matmul(
                        ps[:],
                        xts1[t][:, mi * 128:(mi + 1) * 128],
                        w_tiles[t][:, 2 * 512:3 * 512],
                        start=(t == 0), stop=(t == KT - 1),
                    )
                evict(ps, mi, 2, MS, nc.sync)

            # ---- superchunks 2..15: steady state ----
            for ms in range(2, NMS):
                xts = []
                for u in range(NP):
                    xtp = xt_pool.tile([128, 2, MS], f16, tag="xt",
                                       name="xt")
                    nc.sync.dma_start(
                        xtp[:],
                        xt_r[:, 2 * u:2 * u + 2, ms * MS:(ms + 1) * MS])
                    xts.append(xtp[:, 0, :])
                    xts.append(xtp[:, 1, :])
                for mi in range(MT):
                    for o in range(OT):
                        ps = psum_pool.tile([128, 512], f32, tag="ps",
                                            name="ps")
                        for t in range(KT):
                            nc.tensor.matmul(
                                ps[:],
                                xts[t][:, mi * 128:(mi + 1) * 128],
                                w_tiles[t][:, o * 512:(o + 1) * 512],
                                start=(t == 0), stop=(t == KT - 1),
                            )
                        evict(ps, mi, o, ms * MS,
                              nc.scalar if ms % 2 else nc.sync)

    if not nc.is_finalized():
        nc.finalize()
    return nc


def _unpack_int4_np(q):
    shifts = (np.arange(PACK, dtype=np.int32) * 4)[None, None, :]
    return ((q[:, :, None] >> shifts) & 0xF).reshape(q.shape[0], -1)


def _shard_inputs(x, qweight, scales, qzeros, bias):
    x = np.asarray(x)
    xt_full = np.ascontiguousarray(x.T)              # [K, M]
    perm = _perm(C_SHARD)
    zq_full = _unpack_int4_np(np.asarray(qzeros))
    scales = np.asarray(scales)
    G = K_FULL // GROUP_SIZE
    # per-group column sums of the first superchunk's x rows (f32 sums)
    xg0 = np.ascontiguousarray(
        x[0:512, :].astype(np.float32).reshape(512, G, GROUP_SIZE)
        .sum(-1).T.astype(np.float16))               # [G, 512]
    in_maps = []
    for c in range(N_CORES):
        so = slice(c * O_SHARD, (c + 1) * O_SHARD)
        sc = slice(c * C_SHARD, (c + 1) * C_SHARD)
        s_p = scales[:, so][:, perm].astype(np.float32)
        zsn = -(zq_full[:, so][:, perm].astype(np.float32) * s_p)
        ssz = np.concatenate([s_p, zsn], axis=1).astype(np.float16)
        in_maps.append({
            "xt": xt_full,
            "qw16": np.ascontiguousarray(
                np.asarray(qweight)[:, sc]).view(np.int16),
            "ssz": np.ascontiguousarray(ssz),
            "xg0": xg0,
            "bias": np.ascontiguousarray(
                np.asarray(bias)[so][perm]).reshape(1, -1),
        })
    return in_maps


_CACHED_NC = None


def kernel(x, qweight, scales, qzeros, bias):
    from concourse.bass_utils import run_bass_kernel_spmd

    global _CACHED_NC
    if _CACHED_NC is None:
        _CACHED_NC = build_nc()
    nc = _CACHED_NC

    in_maps = _shard_inputs(x, qweight, scales, qzeros, bias)
    res = run_bass_kernel_spmd(nc, in_maps, core_ids=list(range(N_CORES)))
    # undo the per-core column permutation while gathering
    perm = _perm(C_SHARD)
    out = np.empty((M_FULL, O_FULL), dtype=np.float16)
    for c in range(N_CORES):
        out[:, c * O_SHARD + perm] = res.results[c]["out"]
    return out


# revision 31
# speedup vs baseline: 1.0072x; 1.0072x over previous
"""AWQ 4-bit quantized linear layer on 8 Trainium2 NeuronCores.

Problem: out = x @ dequant(qweight, scales, qzeros) + bias
  x [8192,4096] f16, qweight [4096,1536] i32 (int4 nibbles), scales [32,12288]
  f16 (group 128), qzeros [32,1536] i32, bias [12288] f16 -> out [8192,12288].

Sharding: tensor-parallel colwise, 8 shards of 1536 out-features, x
replicated (host-transposed); host concatenates; no collectives.

The kernel is PE-bound: 6144 bf16 matmuls (K=128, N=512) at the 216 ns
issue floor = 1.327 ms. The structure exists to keep the PE fed from the
first microsecond despite the dequant startup being DMA-bound (aggregate
HBM-DMA cap ~343 GB/s/core; the per-k-tile [scale|zero] row broadcasts are
128x-fanout traffic):
  1. Host layout prep: qweight nibbles widened to uint8 (no arithmetic),
     x pre-transposed, metadata rows [s | -z*s] precomputed (0.4% of
     bytes), plus xg0 = per-group column sums of the first 512 x rows.
  2. Wave 1 dequant produces w' = wq*s ONLY (zero-point deferred), halving
     broadcast traffic in the critical window: per k-tile pair one qw DMA,
     one s-row broadcast, uint8->f16 cast on ACT, one DVE mult. W resident.
  3. Superchunk 0 runs k-PHASED against the dequant frontier (phase A: 8
     psum accumulators over o=0,1; phase B: o=2, t-outer). Each
     accumulator gets one extra K=32 matmul x_groupsums @ (-z*s) that
     exactly cancels the deferred zero-point term (out = x@w' - xg@zs).
  4. Wave 2 (during phase B, on the now-idle rings) broadcasts -z*s and
     adds it into W in place; phase B is t-outer so each w-tile's last
     un-subtracted read completes early. Superchunk 1 is also k-phased so
     it consumes w-tiles no faster than the subs complete. Superchunks
     2..15 then run at the full 216 ns/MM rate with fully-dequanted W.
"""

import sys

for p in ("/opt/trn_rl_repo", "/opt/pypackages"):
    if p not in sys.path:
        sys.path.insert(0, p)

import numpy as np

import concourse.bacc as bacc
import concourse.bass as bass
import concourse.mybir as mybir
from concourse.tile import TileContext

f16 = mybir.dt.float16
f32 = mybir.dt.float32
u8 = mybir.dt.uint8
Alu = mybir.AluOpType

N_CORES = 8
M_FULL, K_FULL, O_FULL = 8192, 4096, 12288
GROUP_SIZE = 128
PACK = 8

O_SHARD = O_FULL // N_CORES        # 1536
C_SHARD = O_SHARD // PACK          # 192


def _perm(C):
    """Column order is natural in this version (host unpacks nibbles)."""
    return np.arange(PACK * C, dtype=np.int64)


def build_nc(M=M_FULL, K=K_FULL, O=O_SHARD, MS=512, xt_bufs=26):
    KT = K // 128                  # k-tiles == quant groups (32)
    G = K // GROUP_SIZE
    assert KT == G
    OT = O // 512                  # 3
    NMS = M // MS                  # 16
    MT = MS // 128                 # 4
    NP = KT // 2                   # 16 k-tile pairs

    nc = bacc.Bacc("TRN2")
    xt_in = nc.dram_tensor("xt", [K, M], f16, kind="ExternalInput")
    qw8 = nc.dram_tensor("qw8", [K, O], u8, kind="ExternalInput")
    # [:, :O] = s, [:, O:] = -z*s (negated so wave-2 and the correction
    # matmul both ADD)
    ssz = nc.dram_tensor("ssz", [G, 2 * O], f16, kind="ExternalInput")
    xg0_in = nc.dram_tensor("xg0", [G, MS], f16, kind="ExternalInput")
    bias = nc.dram_tensor("bias", [1, O], f16, kind="ExternalInput")
    out = nc.dram_tensor("out", [M, O], f16, kind="ExternalOutput")

    with TileContext(nc) as tc:
        with (
            tc.tile_pool(name="wres", bufs=NP) as w_pool,
            tc.tile_pool(name="xt", bufs=xt_bufs) as xt_pool,
            tc.tile_pool(name="qwc", bufs=2) as qwc_pool,
            tc.tile_pool(name="bc", bufs=2) as bc_pool,
            tc.tile_pool(name="meta", bufs=1) as meta_pool,
            tc.tile_pool(name="obuf", bufs=4) as o_pool,
            tc.tile_pool(name="psum", bufs=8, space="PSUM") as psum_pool,
        ):
            qw_r = qw8.rearrange("(t p) c -> p t c", p=128)
            xt_r = xt_in.rearrange("(t p) m -> p t m", p=128)

            # ---- wave 1: w' = wq * s, resident; zero-point deferred ----
            w_tiles = []
            w_pairs = []
            xts0 = []
            for u in range(NP):
                qw_c = qwc_pool.tile([128, 2, O], u8, tag="qwc", name="qwc")
                nc.sync.dma_start(qw_c[:], qw_r[:, 2 * u:2 * u + 2, :])
                xtp = xt_pool.tile([128, 2, MS], f16, tag="xt", name="xt")
                nc.sync.dma_start(xtp[:], xt_r[:, 2 * u:2 * u + 2, 0:MS])
                xts0.append(xtp[:, 0, :])
                xts0.append(xtp[:, 1, :])
                s_b = bc_pool.tile([128, 2, O], f16, tag="sb", name="s_b")
                nc.scalar.dma_start(
                    s_b[:], ssz[2 * u:2 * u + 2, :O].partition_broadcast(128))
                wq_f = bc_pool.tile([128, 2, O], f16, tag="wqf", name="wqf")
                nc.scalar.copy(wq_f[:], qw_c[:])
                w2 = w_pool.tile([128, 2, O], f16, tag="w", name="w")
                nc.vector.tensor_tensor(w2[:], wq_f[:], s_b[:], Alu.mult)
                w_pairs.append(w2)
                for v in range(2):
                    w_tiles.append(w2[:, v, :])
                if u == 2:
                    # off the startup critical path, before first evict/corr
                    bias_b = meta_pool.tile([128, O], f16, tag="biasb")
                    nc.scalar.dma_start(
                        bias_b[:], bias[0, :].partition_broadcast(128))
                    xg0_sb = meta_pool.tile([G, MS], f16, tag="xg0")
                    nc.scalar.dma_start(xg0_sb[:], xg0_in[:, :])
                    zsn_sb = meta_pool.tile([G, O], f16, tag="zsn")
                    nc.scalar.dma_start(zsn_sb[:], ssz[:, O:])

            def corr_mm(ps, mi, o):
                """K=32 matmul adding -(xg0 @ z*s): cancels the deferred
                zero-point for superchunk 0. Ends the accumulation group."""
                nc.tensor.matmul(
                    ps[:],
                    xg0_sb[:, mi * 128:(mi + 1) * 128],
                    zsn_sb[:, o * 512:(o + 1) * 512],
                    start=False, stop=True, skip_group_check=True,
                )

            def evict(ps, mi, o, ms_base, ring):
                ob = o_pool.tile([128, 512], f16, tag="ob", name="ob")
                nc.scalar.copy(ob[:], ps[:])
                nc.vector.tensor_tensor(
                    ob[:], ob[:], bias_b[:, o * 512:(o + 1) * 512], Alu.add)
                m0 = ms_base + mi * 128
                ring.dma_start(
                    out[m0:m0 + 128, o * 512:(o + 1) * 512], ob[:])

            # ---- superchunk 0, phase A: o=0,1 k-phased on 8 banks ----
            psA = []
            for mi in range(MT):
                for o in range(2):
                    ps = psum_pool.tile([128, 512], f32, tag="ps", name="ps")
                    psA.append((mi, o, ps))
            for t in range(KT):
                for mi, o, ps in psA:
                    nc.tensor.matmul(
                        ps[:],
                        xts0[t][:, mi * 128:(mi + 1) * 128],
                        w_tiles[t][:, o * 512:(o + 1) * 512],
                        start=(t == 0), stop=False,
                        skip_group_check=True,
                    )
            for mi, o, ps in psA:
                corr_mm(ps, mi, o)
                evict(ps, mi, o, 0, nc.sync)

            # wave-2 zs broadcast triggers prefire here so they stream on
            # the ACT ring DURING phase B instead of queueing behind its
            # evict copies (engine queues are strict FIFO); the 2-buffer
            # pool gates them, and since phase B is t-outer its w-tile
            # reads release the wave-2 subs (and thus these buffers) early
            zs_bufs = []
            for u in range(NP):
                zs_b = bc_pool.tile([128, 2, O], f16, tag="zsb",
                                    name="zs_b")
                nc.scalar.dma_start(
                    zs_b[:], ssz[2 * u:2 * u + 2, O:].partition_broadcast(128))
                zs_bufs.append(zs_b)

            # ---- superchunk 0, phase B: o=2, t-outer so each w-tile's
            # last pre-subtract read completes early ----
            psB = []
            for mi in range(MT):
                ps = psum_pool.tile([128, 512], f32, tag="ps", name="ps")
                psB.append((mi, ps))
            for t in range(KT):
                for mi, ps in psB:
                    nc.tensor.matmul(
                        ps[:],
                        xts0[t][:, mi * 128:(mi + 1) * 128],
                        w_tiles[t][:, 2 * 512:3 * 512],
                        start=(t == 0), stop=False,
                        skip_group_check=True,
                    )
            for mi, ps in psB:
                corr_mm(ps, mi, 2)
                evict(ps, mi, 2, 0, nc.sync)

            # ---- wave 2: w += (-z*s) using the prefired broadcasts;
            # superchunk-1 x tiles interleave on the SP ring ----
            xts1 = []
            for u in range(NP):
                xtp = xt_pool.tile([128, 2, MS], f16, tag="xt", name="xt")
                nc.sync.dma_start(xtp[:], xt_r[:, 2 * u:2 * u + 2, MS:2 * MS])
                xts1.append(xtp[:, 0, :])
                xts1.append(xtp[:, 1, :])
                nc.vector.tensor_tensor(
                    w_pairs[u][:], w_pairs[u][:], zs_bufs[u][:], Alu.add)

            # ---- superchunk 1: k-phased (consumes w no faster than the
            # wave-2 subs complete), then o=2 normally ----
            psC = []
            for mi in range(MT):
                for o in range(2):
                    ps = psum_pool.tile([128, 512], f32, tag="ps", name="ps")
                    psC.append((mi, o, ps))
            for t in range(KT):
                for mi, o, ps in psC:
                    nc.tensor.matmul(
                        ps[:],
                        xts1[t][:, mi * 128:(mi + 1) * 128],
                        w_tiles[t][:, o * 512:(o + 1) * 512],
                        start=(t == 0), stop=(t == KT - 1),
                        skip_group_check=True,
                    )
            for mi, o, ps in psC:
                evict(ps, mi, o, MS, nc.sync)
            for mi in range(MT):
                ps = psum_pool.tile([128, 512], f32, tag="ps", name="ps")
                for t in range(KT):
                    nc.tensor.matmul(
                        ps[:],
                        xts1[t][:, mi * 128:(mi + 1) * 128],
                        w_tiles[t][:, 2 * 512:3 * 512],
                        start=(t == 0), stop=(t == KT - 1),
                    )
                evict(ps, mi, 2, MS, nc.sync)

            # ---- superchunks 2..15: steady state ----
            for ms in range(2, NMS):
                xts = []
                for u in range(NP):
                    xtp = xt_pool.tile([128, 2, MS], f16, tag="xt",
                                       name="xt")
                    nc.sync.dma_start(
                        xtp[:],
                        xt_r[:, 2 * u:2 * u + 2, ms * MS:(ms + 1) * MS])
                    xts.append(xtp[:, 0, :])
                    xts.append(xtp[:, 1, :])
                for mi in range(MT):
                    for o in range(OT):
                        ps = psum_pool.tile([128, 512], f32, tag="ps",
                                            name="ps")
                        for t in range(KT):
                            nc.tensor.matmul(
                                ps[:],
                                xts[t][:, mi * 128:(mi + 1) * 128],
                                w_tiles[t][:, o * 512:(o + 1) * 512],
                                start=(t == 0), stop=(t == KT - 1),
                            )
                        evict(ps, mi, o, ms * MS,
                              nc.scalar if ms % 2 else nc.sync)

    if not nc.is_finalized():
        nc.finalize()
    return nc


def _unpack_int4_np(q):
    shifts = (np.arange(PACK, dtype=np.int32) * 4)[None, None, :]
    return ((q[:, :, None] >> shifts) & 0xF).reshape(q.shape[0], -1)


def _shard_inputs(x, qweight, scales, qzeros, bias):
    x = np.asarray(x)
    xt_full = np.ascontiguousarray(x.T)              # [K, M]
    qw8_full = _unpack_int4_np(np.asarray(qweight)).astype(np.uint8)
    zq_full = _unpack_int4_np(np.asarray(qzeros))
    scales = np.asarray(scales)
    G = K_FULL // GROUP_SIZE
    # per-group column sums of the first superchunk's x rows (f32 sums)
    xg0 = np.ascontiguousarray(
        x[0:512, :].astype(np.float32).reshape(512, G, GROUP_SIZE)
        .sum(-1).T.astype(np.float16))               # [G, 512]
    in_maps = []
    for c in range(N_CORES):
        so = slice(c * O_SHARD, (c + 1) * O_SHARD)
        s_p = scales[:, so].astype(np.float32)
        zsn = -(zq_full[:, so].astype(np.float32) * s_p)
        ssz = np.concatenate([s_p, zsn], axis=1).astype(np.float16)
        in_maps.append({
            "xt": xt_full,
            "qw8": np.ascontiguousarray(qw8_full[:, so]),
            "ssz": np.ascontiguousarray(ssz),
            "xg0": xg0,
            "bias": np.ascontiguousarray(
                np.asarray(bias)[so]).reshape(1, -1),
        })
    return in_maps


_CACHED_NC = None


def kernel(x, qweight, scales, qzeros, bias):
    from concourse.bass_utils import run_bass_kernel_spmd

    global _CACHED_NC
    if _CACHED_NC is None:
        _CACHED_NC = build_nc()
    nc = _CACHED_NC

    in_maps = _shard_inputs(x, qweight, scales, qzeros, bias)
    res = run_bass_kernel_spmd(nc, in_maps, core_ids=list(range(N_CORES)))
    out = np.empty((M_FULL, O_FULL), dtype=np.float16)
    for c in range(N_CORES):
        out[:, c * O_SHARD:(c + 1) * O_SHARD] = res.results[c]["out"]
    return out
